# revision 3
# baseline (speedup 1.0000x reference)
"""TRN2 kernel for chained bilinear grid lookups (embedding_lookup problem).

Strategy: data-parallel over points (8 cores).  Each bilinear stage is
computed on-device as a "tent basis" matmul: for a 128x128 table block S,
    out[l] = sum_{p,q} relu(1-|su-p|) * relu(1-|sv-q|) * sigmoid(S)[p,q,l]
which equals bilinear interpolation with per-corner sigmoid.  The host
groups points by 127x127 table block (pure data layout), the device does
all per-point math: tent weights on ScalarE, u-contraction + partition
reduce on TensorE, v-weighting on VectorE.  Stage-1 keys come back to the
host, are re-grouped for the 520x520 table, and stage 2 runs the same
kernel structure.
"""
import sys
sys.path.insert(0, "/opt/trn_rl_repo")
sys.path.insert(0, "/root/problem")
import numpy as np

N_CORES = 8
F = 512
BLK = 127  # table-block stride (128-row stationary, 1 row shared)

_kern_cache = {}


def _sigmoid(x):
    return (1.0 / (1.0 + np.exp(-x.astype(np.float32), dtype=np.float32))
            ).astype(np.float32)


def _prep_table(tab):
    """tab [U, V, L] f32 -> sigmoid'd blocked stationaries
    [nbu*nbv, 128, L*128] f32 (wrap-duplicated, block stride BLK)."""
    U, V, L = tab.shape
    nbu = (U - 1) // BLK + 1
    nbv = (V - 1) // BLK + 1
    S = _sigmoid(tab)
    out = np.empty((nbu * nbv, 128, L * 128), np.float32)
    ar = np.arange(128)
    for bu in range(nbu):
        rows = (BLK * bu + ar) % U
        Su = S[rows]  # [128, V, L]
        for bv in range(nbv):
            cols = (BLK * bv + ar) % V
            blkS = Su[:, cols, :]  # [128u, 128v, L]
            b = bu * nbv + bv
            for l in range(L):
                out[b, :, l * 128:(l + 1) * 128] = blkS[:, :, l]
    return out, nbu, nbv


def _get_kernel(nbins, cap, L):
    key = (nbins, cap, L)
    if key not in _kern_cache:
        from stage_kernel import build_stage
        _kern_cache[key] = build_stage(nbins, cap, L, n_cores=N_CORES)
    return _kern_cache[key]


def _consts():
    cst = np.zeros((128, 2), np.float32)
    cst[:, 0] = -np.arange(128)
    cst[:, 1] = 1.0
    sel = np.zeros((128, 256), np.float32)
    sel[:, 128] = 1.0
    return cst, sel


def _run_stage(su_l, sv_l, tabblk, nbu, nbv, L):
    """su_l/sv_l: lists (len 8) of [NS] f32 global scaled coords.
    Returns list of [L, NS] f32 results."""
    from concourse.bass_utils import run_bass_kernel_spmd
    nbins = nbu * nbv
    gpb_cap_inputs = []
    orders, slots, caps = [], [], []
    for c in range(N_CORES):
        su, sv = su_l[c], sv_l[c]
        bu = np.floor(su).astype(np.int64) // BLK
        bv = np.floor(sv).astype(np.int64) // BLK
        binid = (bu * nbv + bv).astype(np.int64)
        order = np.argsort(binid, kind="stable")
        counts = np.bincount(binid, minlength=nbins)
        cumstart = np.concatenate([[0], np.cumsum(counts)])
        sorted_bin = binid[order]
        rank = np.arange(len(su)) - cumstart[sorted_bin]
        orders.append(order)
        slots.append((sorted_bin, rank))
        caps.append(counts.max())
        gpb_cap_inputs.append((su - (BLK * bu).astype(np.float32),
                               sv - (BLK * bv).astype(np.float32), binid))
    cap = int(F * ((max(caps) + F - 1) // F))
    cap = max(cap, F)
    (nc, m) = _get_kernel(nbins, cap, L)
    gpb, gpc = m["gpb"], m["gpc"]
    cst, sel = _consts()
    in_maps = []
    slotidx = []
    for c in range(N_CORES):
        su_loc, sv_loc, binid = gpb_cap_inputs[c]
        order = orders[c]
        sorted_bin, rank = slots[c]
        slot = sorted_bin * cap + rank
        pts = np.full((m["g"], 2 * F), 63.5, np.float32)
        fsu = np.full(m["g"] * F, 63.5, np.float32)
        fsv = np.full(m["g"] * F, 63.5, np.float32)
        # slot s of bin grid -> row g = s//F, col = s%F
        fsu[slot] = su_loc[order]
        fsv[slot] = sv_loc[order]
        pts[:, 0:F] = fsu.reshape(m["g"], F)
        pts[:, F:2 * F] = fsv.reshape(m["g"], F)
        in_maps.append({"pts": pts, "tab": tabblk, "consts": cst,
                        "sel": sel})
        slotidx.append((order, slot))
    res = run_bass_kernel_spmd(nc, in_maps,
                               core_ids=list(range(N_CORES)))
    outs = []
    for c in range(N_CORES):
        order, slot = slotidx[c]
        o = res.results[c]["out"]  # [ngrp, 128, F]
        g = slot // F
        col = slot % F
        r = np.empty((L, len(order)), np.float32)
        for l in range(L):
            vals = o[g // gpc, (g % gpc) * L + l, col]
            tmp = np.empty(len(order), np.float32)
            tmp[order] = vals
            r[l] = tmp
        outs.append(r)
    return outs


def kernel(x, grid1_table, grid0_table):
    N = x.shape[0]
    NS = N // N_CORES
    U1, V1, L1 = grid1_table.shape
    U0, V0, L0 = grid0_table.shape

    tab1, nbu1, nbv1 = _prep_table(grid1_table)
    tab0, nbu0, nbv0 = _prep_table(grid0_table)

    su_l, sv_l = [], []
    for c in range(N_CORES):
        xs = x[c * NS:(c + 1) * NS]
        su_l.append((xs[:, 0] * np.float32(U1)).astype(np.float32))
        sv_l.append((xs[:, 1] * np.float32(V1)).astype(np.float32))

    keys = _run_stage(su_l, sv_l, tab1, nbu1, nbv1, L1)

    su2_l = [(k[0] * np.float32(U0)).astype(np.float32) for k in keys]
    sv2_l = [(k[1] * np.float32(V0)).astype(np.float32) for k in keys]

    outs = _run_stage(su2_l, sv2_l, tab0, nbu0, nbv0, L0)

    return np.concatenate([o.T for o in outs], axis=0)
